# revision 22
# baseline (speedup 1.0000x reference)
"""CenterLoss kernel for 8 Trainium2 NeuronCores (Bass/Tile).

Reference computation:
    label = argmax(predicts, axis=-1)            # [N], N = 32*256 = 8192
    d_n   = ||features_n - centers[label_n]||^2  # [N]
    loss  = (sum_n clip(d_n, EPS, INF) + N*(C-1)*EPS) / N

Sharding: data-parallel over the flattened N axis — 1024 rows per core,
centers replicated. The scalar "all-reduce" is done host-side.

Per-core structure (v2 — tail-free streaming):
  Columns are split A=[0:6125] (49 chunks of 125) and B=[6125:6625] (500).
  * A side: stream [128, 6125] tiles, hierarchical argmax on DVE
    (chunk reduce_max -> Max8 -> FindIndex8), regather the winning chunk
    via indirect DMA, FindIndex8 again -> label_a, gather centers[label_a],
    d_a = ||f - c||^2 via GpSimd sub + ScalarE Square+accum.  All of this
    overlaps the predicts stream.
  * B side: while streaming, idle PE precomputes
    Dpre[n, j] = -2 f_n . c_j + ||c_j||^2 for the 500 B-columns
    (f32 matmul [97,128]^T @ [97,500] -> PSUM, one bank per tile).
    The 8 B pieces [128, 500] are loaded LAST in the stream; resolving a
    tile's B side is then just reduce_max (GpSimd) + one fused DVE
    scalar_tensor_tensor (is_equal mask * Dpre, row-sum, init fsq) and a
    2-op select against the A candidate.  The post-stream tail is ~3us
    instead of ~20us (the old regather+gather round trips for the last
    tiles).
"""

import numpy as np

import concourse.bacc as bacc
import concourse.bass as bass
import concourse.mybir as mybir
from concourse import tile
from concourse.bass_utils import run_bass_kernel_spmd

B, T, D, C = 32, 256, 96, 6625
N = B * T                  # 8192 rows total
NCORES = 8
NS = N // NCORES           # 1024 rows per core
P = 128                    # SBUF partitions
NT = NS // P               # 8 predicts tiles per core
CW = 125                   # chunk width
NCH = 53                   # chunks per full row (for the predflat view)
CA = 6125                  # A-side columns = 49 chunks
NCA = CA // CW             # 49
CB = C - CA                # 500 B-side columns
EPS = 1e-7

# test.py toggles these module-level knobs; the grading harness just calls
# kernel(**inputs) and gets the defaults.
TRACE = False
TRACE_KWARGS = {}
LAST_RESULTS = None

# iotas[p, t] = (t*P + p) * NCH — base chunk index per (partition, tile)
_IOTAS = np.ascontiguousarray(
    ((np.arange(NT)[None, :] * P + np.arange(P)[:, None]) * NCH).astype(np.int32)
)


def _build():
    nc = bacc.Bacc("TRN2", num_devices=NCORES)
    f32 = mybir.dt.float32
    u32 = mybir.dt.uint32
    pred = nc.dram_tensor("predicts", [NS, C], f32, kind="ExternalInput").ap()
    # features pre-transposed host-side to [P, NT*D] (partition-major)
    feat = nc.dram_tensor("features", [P, NT * D], f32, kind="ExternalInput").ap()
    # features again, as matmul lhsT: [97, NT*P]; row 96 is all-ones
    ftaug = nc.dram_tensor("ftaug", [D + 1, NT * P], f32, kind="ExternalInput").ap()
    # B-side centers, augmented: [97, 500]; rows 0..95 = -2*C_B^T, row 96 = ||c||^2
    cbaug = nc.dram_tensor("cbaug", [D + 1, CB], f32, kind="ExternalInput").ap()
    iot = nc.dram_tensor("iotas", [P, NT], mybir.dt.int32, kind="ExternalInput").ap()
    cent = nc.dram_tensor("centers", [C, D], f32, kind="ExternalInput").ap()
    dist = nc.dram_tensor("dists", [P, NT], f32, kind="ExternalOutput").ap()

    # flat chunk view for the winning-chunk regather: row r, chunk k lives at
    # predflat[r * NCH + k, :]
    predflat = pred.rearrange("n (k q) -> (n k) q", q=CW)

    with tile.TileContext(nc) as tc:
        with (
            tc.tile_pool(name="pred", bufs=5) as pp,
            tc.tile_pool(name="small", bufs=3) as sp,
            tc.tile_pool(name="persist", bufs=1) as ps,
            tc.tile_pool(name="dpre", bufs=1, space="PSUM") as psum,
        ):
            # aux tiles — loaded on the otherwise-idle SWDGE q0.  Keep them OFF
            # the HWDGE queues: a contiguous DRAM region's descriptors all land
            # on ONE DMA engine (~8 B/ns drip) and monopolize the ring,
            # starving the predicts stream.  Ordered by first consumer.
            ftile_flat = ps.tile([P, NT * D], f32)
            ftaug_flat = ps.tile([D + 1, NT * P], f32)
            cbaug_sb = ps.tile([D + 1, CB], f32)
            iotas = ps.tile([P, NT], mybir.dt.int32)
            nc.gpsimd.dma_start(iotas[:], iot[:])
            nc.gpsimd.dma_start(ftile_flat[:], feat[:])
            nc.gpsimd.dma_start(cbaug_sb[:], cbaug[:])
            nc.gpsimd.dma_start(ftaug_flat[:], ftaug[:])

            finb = ps.tile([P, NT, CB], f32)      # B pieces (loaded at the end)
            top8s = ps.tile([P, NT, 8], f32)      # per-tile A-side top8
            ctile = ps.tile([P, NT, D], f32)      # gathered centers (A side)
            diff = ps.tile([P, NT, D], f32)
            laba = ps.tile([P, NT], u32)
            offs = ps.tile([P, NT], u32)
            d_a = ps.tile([P, NT], f32)           # A-side ||f - c||^2
            fsq = ps.tile([P, NT], f32)           # ||f||^2
            vb = ps.tile([P, NT], f32)            # B-side row max
            db = ps.tile([P, NT], f32)            # sum(mask * Dpre)
            ge = ps.tile([P, NT], f32)            # (vb > va)
            delta = ps.tile([P, NT], f32)         # d_b - d_a
            dist2 = ps.tile([P, NT], f32)         # final per-row distance
            gath = ps.tile([P, NT, CW], f32)      # regathered winning chunks
            cidx8s = ps.tile([P, NT, 8], u32)

            # Dpre PSUM banks: one [128, 500] f32 matmul result per tile,
            # computed by the otherwise-idle PE as soon as ftaug/cbaug land
            dpre = [psum.tile([P, 512], f32, name=f"dpre{t}") for t in range(NT)]
            for tm in range(NT):
                nc.tensor.matmul(
                    dpre[tm][:, 0:CB],
                    ftaug_flat[:, tm * P : (tm + 1) * P],
                    cbaug_sb[:],
                    start=True,
                    stop=True,
                )

            def phase2(t):
                """A-side gather-dependent work for tile t (call >=2 tiles later)."""
                widx8 = sp.tile([P, 8], u32, tag="widx8")
                nc.vector.max_index(
                    out=widx8[:], in_max=top8s[:, t, :], in_values=gath[:, t, :]
                )
                # label_a = cidx * CW + widx
                nc.vector.tensor_scalar(
                    laba[:, t : t + 1], cidx8s[:, t, 0:1], float(CW), None,
                    op0=mybir.AluOpType.mult,
                )
                nc.vector.tensor_add(
                    laba[:, t : t + 1], laba[:, t : t + 1], widx8[:, 0:1]
                )
                # centers[label_a] gather: one 384B row per partition
                nc.gpsimd.indirect_dma_start(
                    out=ctile[:, t, :],
                    out_offset=None,
                    in_=cent[:],
                    in_offset=bass.IndirectOffsetOnAxis(ap=laba[:, t : t + 1], axis=0),
                )
                nc.gpsimd.tensor_tensor(
                    diff[:, t, :], ftile_flat[:, t * D : (t + 1) * D], ctile[:, t, :],
                    op=mybir.AluOpType.subtract,
                )
                sqs = sp.tile([P, D], f32, tag="sqs")
                nc.scalar.activation(
                    sqs[:], diff[:, t, :], mybir.ActivationFunctionType.Square,
                    accum_out=d_a[:, t : t + 1],
                )
                # ||f||^2 for tile t, same 2-behind cadence (ftile has long
                # landed, so this never blocks the scalar SEQ's DMA issues)
                sqf = sp.tile([P, D], f32, tag="sqf")
                nc.scalar.activation(
                    sqf[:], ftile_flat[:, t * D : (t + 1) * D],
                    mybir.ActivationFunctionType.Square,
                    accum_out=fsq[:, t : t + 1],
                )

            HALF = 25 * CW  # 3125; A halves are 25 + 24 chunks
            for t in range(NT):
                pt = pp.tile([P, CA], f32, tag="pt")
                cm = sp.tile([P, NCA], f32, tag="cm")
                rows = pred[t * P : (t + 1) * P, :]
                if t == NT - 1:
                    # quarter-split the last tile: its final reduce piece is
                    # small, so the post-stream argmax chain starts ASAP
                    bounds = [0, 13, 25, 38, 49]
                    for j in range(4):
                        lo, hi = bounds[j] * CW, bounds[j + 1] * CW
                        eng = nc.sync if j % 2 == 0 else nc.scalar
                        eng.dma_start(pt[:, lo:hi], rows[:, lo:hi])
                        nc.vector.reduce_max(
                            cm[:, bounds[j] : bounds[j + 1]],
                            pt[:, lo:hi].rearrange("p (k q) -> p k q", q=CW),
                            axis=mybir.AxisListType.X,
                        )
                else:
                    nc.sync.dma_start(pt[:, :HALF], rows[:, :HALF])
                    nc.scalar.dma_start(pt[:, HALF:], rows[:, HALF:CA])
                    nc.vector.reduce_max(
                        cm[:, :25],
                        pt[:, :HALF].rearrange("p (k q) -> p k q", q=CW),
                        axis=mybir.AxisListType.X,
                    )
                    nc.vector.reduce_max(
                        cm[:, 25:],
                        pt[:, HALF:].rearrange("p (k q) -> p k q", q=CW),
                        axis=mybir.AxisListType.X,
                    )
                nc.vector.max(out=top8s[:, t, :], in_=cm[:])
                nc.vector.max_index(
                    out=cidx8s[:, t, :], in_max=top8s[:, t, :], in_values=cm[:]
                )
                nc.vector.tensor_add(
                    offs[:, t : t + 1], iotas[:, t : t + 1], cidx8s[:, t, 0:1]
                )
                if t >= 2:
                    phase2(t - 2)
                nc.gpsimd.indirect_dma_start(
                    out=gath[:, t, :],
                    out_offset=None,
                    in_=predflat,
                    in_offset=bass.IndirectOffsetOnAxis(ap=offs[:, t : t + 1], axis=0),
                )


            phase2(NT - 2)

            # B pieces stream in AFTER every A piece (queue program order)
            for t in range(NT):
                eng = nc.sync if t % 2 == 0 else nc.scalar
                eng.dma_start(
                    finb[:, t, :], pred[t * P : (t + 1) * P, CA:C]
                )

            phase2(NT - 1)

            # B-side resolution as the pieces land (free-axis reduce is
            # DVE-only; the whole B chain is ~0.9us of DVE per tile)
            for t in range(NT):
                nc.vector.reduce_max(
                    vb[:, t : t + 1],
                    finb[:, t, :].rearrange("p (k q) -> p k q", q=CB),
                    axis=mybir.AxisListType.X,
                )
                # fused: mask = (finb == vb); db = sum(mask * Dpre)
                msk = sp.tile([P, CB], f32, tag="msk")
                nc.vector.scalar_tensor_tensor(
                    out=msk[:],
                    in0=finb[:, t, :],
                    scalar=vb[:, t : t + 1],
                    in1=dpre[t][:, 0:CB],
                    op0=mybir.AluOpType.is_equal,
                    op1=mybir.AluOpType.mult,
                    accum_out=db[:, t : t + 1],
                )
                # d_b = fsq + db ; select: d = (vb > va) ? d_b : d_a
                nc.vector.tensor_tensor(
                    ge[:, t : t + 1], vb[:, t : t + 1], top8s[:, t, 0:1],
                    op=mybir.AluOpType.is_gt,
                )
                nc.vector.scalar_tensor_tensor(
                    out=delta[:, t : t + 1],
                    in0=db[:, t : t + 1],
                    scalar=fsq[:, t : t + 1],
                    in1=d_a[:, t : t + 1],
                    op0=mybir.AluOpType.add,
                    op1=mybir.AluOpType.subtract,
                )
                nc.vector.scalar_tensor_tensor(
                    out=dist2[:, t : t + 1],
                    in0=delta[:, t : t + 1],
                    scalar=ge[:, t : t + 1],
                    in1=d_a[:, t : t + 1],
                    op0=mybir.AluOpType.mult,
                    op1=mybir.AluOpType.add,
                )

            nc.sync.dma_start(dist[:], dist2[:])
    nc.compile()
    return nc


def kernel(features, predicts, centers):
    global LAST_RESULTS
    feats = np.ascontiguousarray(np.asarray(features).reshape(N, D), dtype=np.float32)
    preds = np.ascontiguousarray(np.asarray(predicts).reshape(N, C), dtype=np.float32)
    cents = np.ascontiguousarray(np.asarray(centers), dtype=np.float32)

    # B-side centers, augmented for the PE: rows 0..95 = -2*C_B^T, row 96 = ||c||^2
    cB = cents[CA:C]                                   # [500, 96]
    cbaug = np.empty((D + 1, CB), dtype=np.float32)
    cbaug[:D, :] = -2.0 * cB.T
    cbaug[D, :] = (cB * cB).sum(axis=1)

    nc = _build()
    in_maps = []
    for i in range(NCORES):
        fshard = feats[i * NS : (i + 1) * NS]          # [1024, 96]
        # [P, NT*D] partition-major layout: row t*128+p -> [p, t*D:(t+1)*D]
        fT = np.ascontiguousarray(
            fshard.reshape(NT, P, D).transpose(1, 0, 2).reshape(P, NT * D)
        )
        # matmul lhsT layout [97, NT*P]: [k, t*128+p] = f[t*128+p, k]; row 96 = 1
        fA = np.empty((D + 1, NT * P), dtype=np.float32)
        fA[:D, :] = fshard.T
        fA[D, :] = 1.0
        in_maps.append(
            {
                "predicts": preds[i * NS : (i + 1) * NS],
                "features": fT,
                "ftaug": np.ascontiguousarray(fA),
                "cbaug": cbaug,
                "centers": cents,
                "iotas": _IOTAS,
            }
        )
    res = run_bass_kernel_spmd(
        nc, in_maps, core_ids=list(range(NCORES)), trace=TRACE, **TRACE_KWARGS
    )
    LAST_RESULTS = res

    total = 0.0
    for r in res.results:
        # EPS clip of the per-row distances happens here as part of the
        # unshard-reduce (only the lower clip can bind)
        total += float(np.maximum(r["dists"], EPS).astype(np.float64).sum())
    total += float(N) * (C - 1) * EPS
    return np.asarray(total / N, dtype=np.float32)


# revision 24
# speedup vs baseline: 1.1962x; 1.1962x over previous
"""CenterLoss kernel for 8 Trainium2 NeuronCores (Bass/Tile).

Reference computation:
    label = argmax(predicts, axis=-1)            # [N], N = 32*256 = 8192
    d_n   = ||features_n - centers[label_n]||^2  # [N]
    loss  = (sum_n clip(d_n, EPS, INF) + N*(C-1)*EPS) / N

Sharding: data-parallel over the flattened N axis — 1024 rows per core,
centers replicated. The scalar "all-reduce" is done host-side.

Per-core structure: stream the [1024, 6625] predicts shard through SBUF in
8 [128, *] tiles; per-row argmax via DVE chunk reduce_max -> Max8 ->
FindIndex8, regather of the winning 125-wide chunk (indirect DMA), second
FindIndex8 -> label, centers[label] gather, d = ||f - c||^2 (GpSimd sub +
ScalarE Square+accum).  All of that pipelines 2 tiles behind the stream.

Tail optimization: the last two tiles (6, 7) only stream columns [0:6125]
("A side"); their last 500 columns ("B side") are loaded at the very END of
the stream.  While streaming, the idle PE precomputes
Dpre[n, j] = -2 f_n . c_j + ||c_j||^2 for those 500 centers (f32 matmul ->
PSUM), so resolving a B side is just reduce_max + one fused
scalar_tensor_tensor (is_equal mask * Dpre, row-sum, +||f||^2) and a 3-op
select against the A candidate — no gather round trips after the stream
ends.  This removes most of the old ~20us post-stream serial tail (the
regather+centers-gather chains of tiles 6/7).
"""

import numpy as np

import concourse.bacc as bacc
import concourse.bass as bass
import concourse.mybir as mybir
from concourse import tile
from concourse.bass_utils import run_bass_kernel_spmd

B, T, D, C = 32, 256, 96, 6625
N = B * T                  # 8192 rows total
NCORES = 8
NS = N // NCORES           # 1024 rows per core
P = 128                    # SBUF partitions
NT = NS // P               # 8 predicts tiles per core
CW = 125                   # chunk width
NCH = 53                   # chunks per full row (predflat view)
CA = 6125                  # A-side columns for the B-split tiles (49 chunks)
NCA = CA // CW             # 49
CB = C - CA                # 500 B-side columns
NB = 2                     # number of B-split tiles (the last NB tiles)
TB0 = NT - NB              # first B-split tile
PAD = 6656                 # aux DRAM row pitch (floats) — a contiguous DRAM
                           # region's HWDGE descriptors all aggregate onto one
                           # DMA engine (~8 B/ns drip); a predicts-like pitch
                           # spreads them across all 16
EPS = 1e-7

# test.py toggles these module-level knobs; the grading harness just calls
# kernel(**inputs) and gets the defaults.
TRACE = False
TRACE_KWARGS = {}
LAST_RESULTS = None

# iotas[p, t] = (t*P + p) * NCH — base chunk index per (partition, tile)
_IOTAS = np.ascontiguousarray(
    ((np.arange(NT)[None, :] * P + np.arange(P)[:, None]) * NCH).astype(np.int32)
)


def _build():
    nc = bacc.Bacc("TRN2", num_devices=NCORES)
    f32 = mybir.dt.float32
    u32 = mybir.dt.uint32
    pred = nc.dram_tensor("predicts", [NS, C], f32, kind="ExternalInput").ap()
    # features arrive host-pre-transposed to [P, NT*D] (partition-major)
    feat = nc.dram_tensor("features", [P, NT * D], f32, kind="ExternalInput").ap()
    # matmul lhsT for the B-split tiles: [97, NB*P]; row 96 is all-ones
    ftaug = nc.dram_tensor("ftaug", [D + 1, PAD], f32, kind="ExternalInput").ap()
    # B-side centers, augmented: rows 0..95 = -2*C_B^T, row 96 = ||c||^2
    cbaug = nc.dram_tensor("cbaug", [D + 1, PAD], f32, kind="ExternalInput").ap()
    iot = nc.dram_tensor("iotas", [P, NT], mybir.dt.int32, kind="ExternalInput").ap()
    cent = nc.dram_tensor("centers", [C, D], f32, kind="ExternalInput").ap()
    dist = nc.dram_tensor("dists", [P, NT], f32, kind="ExternalOutput").ap()

    # flat chunk view for the winning-chunk regather: row r, chunk k lives at
    # predflat[r * NCH + k, :]
    predflat = pred.rearrange("n (k q) -> (n k) q", q=CW)

    with tile.TileContext(nc) as tc:
        with (
            tc.tile_pool(name="pred", bufs=5) as pp,
            tc.tile_pool(name="small", bufs=3) as sp,
            tc.tile_pool(name="persist", bufs=1) as ps,
            tc.tile_pool(name="dpre", bufs=1, space="PSUM") as psum,
        ):
            ftile = ps.tile([P, NT, D], f32)
            nc.gpsimd.dma_start(ftile[:], feat.rearrange("p (t d) -> p t d", d=D))
            iotas = ps.tile([P, NT], mybir.dt.int32)
            nc.gpsimd.dma_start(iotas[:], iot[:])

            ftaug_flat = ps.tile([D + 1, NB * P], f32)
            cbaug_sb = ps.tile([D + 1, CB], f32)
            finb = ps.tile([P, NB, CB], f32)      # B pieces (loaded at the end)
            ctile = ps.tile([P, NT, D], f32)
            gath = ps.tile([P, NT, CW], f32)
            offs = ps.tile([P, NT], u32)
            laba = ps.tile([P, NT], u32)
            diff = ps.tile([P, NT, D], f32)
            sq = ps.tile([P, NT, D], f32)
            d2 = ps.tile([P, NT], f32)
            fsq = ps.tile([P, NB], f32)           # ||f||^2, B-split tiles only
            vb = ps.tile([P, NB], f32)
            db = ps.tile([P, NB], f32)
            geb = ps.tile([P, NB], f32)
            deltab = ps.tile([P, NB], f32)

            # persistent per-tile top8/cidx8 so phase-2 work can run 2 tiles
            # behind phase 1 (software pipeline: gather round trips never
            # stall the DVE stream)
            top8s = ps.tile([P, NT, 8], f32)
            cidx8s = ps.tile([P, NT, 8], u32)

            dpre = [psum.tile([P, 512], f32, name=f"dpre{i}") for i in range(NB)]

            def phase2(t):
                """tile t's gather-dependent work; call >=2 tiles later."""
                widx8 = sp.tile([P, 8], u32, tag="widx8")
                nc.vector.max_index(
                    out=widx8[:], in_max=top8s[:, t, :], in_values=gath[:, t, :]
                )
                # label = cidx * CW + widx
                nc.vector.tensor_scalar(
                    laba[:, t : t + 1], cidx8s[:, t, 0:1], float(CW), None,
                    op0=mybir.AluOpType.mult,
                )
                nc.vector.tensor_add(
                    laba[:, t : t + 1], laba[:, t : t + 1], widx8[:, 0:1]
                )
                # centers[label] gather: one 384B row per partition
                nc.gpsimd.indirect_dma_start(
                    out=ctile[:, t, :],
                    out_offset=None,
                    in_=cent[:],
                    in_offset=bass.IndirectOffsetOnAxis(ap=laba[:, t : t + 1], axis=0),
                )
                # distance: subtract on GpSimd (a DVE sub here would stall the
                # DVE FIFO on the centers-gather round trip), square+row-sum
                # on ScalarE
                nc.gpsimd.tensor_tensor(
                    diff[:, t, :], ftile[:, t, :], ctile[:, t, :],
                    op=mybir.AluOpType.subtract,
                )
                nc.scalar.activation(
                    sq[:, t, :], diff[:, t, :], mybir.ActivationFunctionType.Square,
                    accum_out=d2[:, t : t + 1],
                )

            HALF = 27 * CW  # full tiles: column split 27 + 26 chunks
            HALFA = 25 * CW  # B-split tiles: 25 + 24 chunks of the 49
            for t in range(NT):
                pt = pp.tile([P, C], f32, tag="pt")
                cm = sp.tile([P, NCH], f32, tag="cm")
                rows = pred[t * P : (t + 1) * P, :]
                ncm = NCH if t < TB0 else NCA
                if t <= 1:
                    # quarter-split the first two tiles: earlier DVE start
                    bounds = [0, 14, 27, 40, 53]
                    for j in range(4):
                        lo, hi = bounds[j] * CW, bounds[j + 1] * CW
                        eng = nc.sync if j % 2 == 0 else nc.scalar
                        eng.dma_start(pt[:, lo:hi], rows[:, lo:hi])
                        nc.vector.reduce_max(
                            cm[:, bounds[j] : bounds[j + 1]],
                            pt[:, lo:hi].rearrange("p (k q) -> p k q", q=CW),
                            axis=mybir.AxisListType.X,
                        )
                    if t == 1:
                        # aux loads for the B side (padded pitch, see PAD note)
                        nc.sync.dma_start(cbaug_sb[:], cbaug[:, 0:CB])
                        nc.scalar.dma_start(ftaug_flat[:], ftaug[:, 0 : NB * P])
                        for i in range(NB):
                            nc.tensor.matmul(
                                dpre[i][:, 0:CB],
                                ftaug_flat[:, i * P : (i + 1) * P],
                                cbaug_sb[:],
                                start=True,
                                stop=True,
                            )
                        # ||f||^2 for the B-split tiles (ftile is landing now;
                        # scalar ring holds ~2 quarter pieces, so a short wait
                        # here cannot starve the queue)
                        for i in range(NB):
                            sqf = sp.tile([P, D], f32, tag="sqf")
                            nc.scalar.activation(
                                sqf[:], ftile[:, TB0 + i, :],
                                mybir.ActivationFunctionType.Square,
                                accum_out=fsq[:, i : i + 1],
                            )
                elif t < TB0:
                    nc.sync.dma_start(pt[:, :HALF], rows[:, :HALF])
                    nc.scalar.dma_start(pt[:, HALF:], rows[:, HALF:])
                    nc.vector.reduce_max(
                        cm[:, :27],
                        pt[:, :HALF].rearrange("p (k q) -> p k q", q=CW),
                        axis=mybir.AxisListType.X,
                    )
                    nc.vector.reduce_max(
                        cm[:, 27:],
                        pt[:, HALF:].rearrange("p (k q) -> p k q", q=CW),
                        axis=mybir.AxisListType.X,
                    )
                elif t == TB0:
                    # B-split tile, halves over the 49 A chunks
                    nc.sync.dma_start(pt[:, :HALFA], rows[:, :HALFA])
                    nc.scalar.dma_start(pt[:, HALFA:CA], rows[:, HALFA:CA])
                    nc.vector.reduce_max(
                        cm[:, :25],
                        pt[:, :HALFA].rearrange("p (k q) -> p k q", q=CW),
                        axis=mybir.AxisListType.X,
                    )
                    nc.vector.reduce_max(
                        cm[:, 25:NCA],
                        pt[:, HALFA:CA].rearrange("p (k q) -> p k q", q=CW),
                        axis=mybir.AxisListType.X,
                    )
                else:
                    # last tile: quarter-split its 49 A chunks so the final
                    # reduce piece is small
                    bounds = [0, 13, 25, 38, 49]
                    for j in range(4):
                        lo, hi = bounds[j] * CW, bounds[j + 1] * CW
                        eng = nc.sync if j % 2 == 0 else nc.scalar
                        eng.dma_start(pt[:, lo:hi], rows[:, lo:hi])
                        nc.vector.reduce_max(
                            cm[:, bounds[j] : bounds[j + 1]],
                            pt[:, lo:hi].rearrange("p (k q) -> p k q", q=CW),
                            axis=mybir.AxisListType.X,
                        )
                nc.vector.max(out=top8s[:, t, :], in_=cm[:, 0:ncm])
                nc.vector.max_index(
                    out=cidx8s[:, t, :], in_max=top8s[:, t, :], in_values=cm[:, 0:ncm]
                )
                nc.vector.tensor_add(
                    offs[:, t : t + 1], iotas[:, t : t + 1], cidx8s[:, t, 0:1]
                )
                if t == NT - 1:
                    # B pieces enter both queues right behind the last A piece
                    for i in range(NB):
                        eng = nc.sync if i % 2 == 0 else nc.scalar
                        eng.dma_start(
                            finb[:, i, :],
                            pred[(TB0 + i) * P : (TB0 + i + 1) * P, CA:C],
                        )
                nc.gpsimd.indirect_dma_start(
                    out=gath[:, t, :],
                    out_offset=None,
                    in_=predflat,
                    in_offset=bass.IndirectOffsetOnAxis(ap=offs[:, t : t + 1], axis=0),
                )
                if t >= 2:
                    phase2(t - 2)

            phase2(NT - 2)
            phase2(NT - 1)

            # B-side resolution for tiles 6, 7: d_b = fsq + sum(mask * Dpre);
            # d = (vb > va) ? d_b : d_a  (strict gt keeps argmax-first ties)
            for i in range(NB):
                t = TB0 + i
                nc.vector.reduce_max(
                    vb[:, i : i + 1],
                    finb[:, i, :].rearrange("p (k q) -> p k q", q=CB),
                    axis=mybir.AxisListType.X,
                )
                msk = sp.tile([P, CB], f32, tag="msk")
                nc.vector.scalar_tensor_tensor(
                    out=msk[:],
                    in0=finb[:, i, :],
                    scalar=vb[:, i : i + 1],
                    in1=dpre[i][:, 0:CB],
                    op0=mybir.AluOpType.is_equal,
                    op1=mybir.AluOpType.mult,
                    accum_out=db[:, i : i + 1],
                )
                nc.vector.tensor_tensor(
                    geb[:, i : i + 1], vb[:, i : i + 1], top8s[:, t, 0:1],
                    op=mybir.AluOpType.is_gt,
                )
                # delta = (db + fsq) - d_a ; d = delta * ge + d_a
                nc.vector.scalar_tensor_tensor(
                    out=deltab[:, i : i + 1],
                    in0=db[:, i : i + 1],
                    scalar=fsq[:, i : i + 1],
                    in1=d2[:, t : t + 1],
                    op0=mybir.AluOpType.add,
                    op1=mybir.AluOpType.subtract,
                )
                nc.vector.scalar_tensor_tensor(
                    out=d2[:, t : t + 1],
                    in0=deltab[:, i : i + 1],
                    scalar=geb[:, i : i + 1],
                    in1=d2[:, t : t + 1],
                    op0=mybir.AluOpType.mult,
                    op1=mybir.AluOpType.add,
                )

            nc.sync.dma_start(dist[:], d2[:])
    nc.compile()
    return nc


def kernel(features, predicts, centers):
    global LAST_RESULTS
    feats = np.ascontiguousarray(np.asarray(features).reshape(N, D), dtype=np.float32)
    preds = np.ascontiguousarray(np.asarray(predicts).reshape(N, C), dtype=np.float32)
    cents = np.ascontiguousarray(np.asarray(centers), dtype=np.float32)

    # B-side centers, augmented for the PE (padded row pitch, see PAD note)
    cB = cents[CA:C]                                   # [500, 96]
    cbaug = np.zeros((D + 1, PAD), dtype=np.float32)
    cbaug[:D, :CB] = -2.0 * cB.T
    cbaug[D, :CB] = (cB * cB).sum(axis=1)

    nc = _build()
    in_maps = []
    for i in range(NCORES):
        fshard = feats[i * NS : (i + 1) * NS]          # [1024, 96]
        # [P, NT*D] partition-major layout: row t*128+p -> [p, t*D:(t+1)*D]
        fT = np.ascontiguousarray(
            fshard.reshape(NT, P, D).transpose(1, 0, 2).reshape(P, NT * D)
        )
        # matmul lhsT for tiles TB0..NT-1: [97, NB*P]; row 96 = 1
        fA = np.zeros((D + 1, PAD), dtype=np.float32)
        fA[:D, : NB * P] = fshard[TB0 * P :].T
        fA[D, : NB * P] = 1.0
        in_maps.append(
            {
                "predicts": preds[i * NS : (i + 1) * NS],
                "features": fT,
                "ftaug": fA,
                "cbaug": cbaug,
                "centers": cents,
                "iotas": _IOTAS,
            }
        )
    res = run_bass_kernel_spmd(
        nc, in_maps, core_ids=list(range(NCORES)), trace=TRACE, **TRACE_KWARGS
    )
    LAST_RESULTS = res

    total = 0.0
    for r in res.results:
        # EPS clip of the per-row distances happens here as part of the
        # unshard-reduce (only the lower clip can bind)
        total += float(np.maximum(r["dists"], EPS).astype(np.float64).sum())
    total += float(N) * (C - 1) * EPS
    return np.asarray(total / N, dtype=np.float32)
